# revision 1
# baseline (speedup 1.0000x reference)
"""Trainium2 Bass kernel for nn_CausalAttention (diff-attention with QK-norm,
RoPE, GQA, tanh soft-cap, causal softmax).

Sharding: 8 cores = (batch b in {0,1}) x (kv-group j in {0..3}).
Each core handles one batch element and the 4 query heads of one kv head.

Precision/perf strategy (fp32 matmuls cost 4 cyc/row on TRN2):
  - big GEMMs (projections, scores, O-proj) run as 3 bf16 matmuls on an
    exact hi/lo bf16 split of each operand (error ~= dropped lo*lo term,
    ~1.6e-5 relative) at 1 cyc/row each
  - attn_diff^T is built by two accumulated fp32 PE transpose-mode matmuls
    (2 cyc/row) of 1/r1-scaled e1 and (-lambda/r2)-scaled e2
  - AV and everything touching attention weights stays fp32
  - RMSNorm rsqrt runs as Newton iteration on DVE so ScalarE only ever
    needs the exp_and_others table (tanh+exp), avoiding table reloads
"""

import os
import sys

import numpy as np

if "/opt/trn_rl_repo" not in sys.path:
    sys.path.insert(0, "/opt/trn_rl_repo")

import ml_dtypes
import concourse.bass as bass
import concourse.mybir as mybir
import concourse.tile as tile
from concourse import bacc
from concourse.bass_utils import run_bass_kernel_spmd
from concourse.masks import make_identity

B, S, D = 2, 2048, 1024
H, KV, HD = 16, 4, 64
G = H // KV          # q heads per kv head (= heads per core)
CAP = 50.0
EPS = 1e-6
SCALE = 1.0 / 8.0    # 1/sqrt(HD)
P = 128
NSB = S // P         # 16 s-blocks
EQ = G * 2 * HD      # 512 q-projection cols per core
EK = 2 * HD          # 128 k-projection cols per core
EV = HD              # 64  v-projection cols per core
EQK = EQ + EK        # 640 cols needing norm+rope
EALL = EQ + EK + EV  # 704 projection cols per core
NG = EQK // HD       # 10 rmsnorm groups
KT = D // P          # 8 contraction tiles
MASK_FILL = -1.0e4   # exp(50 * -1e4) == 0 in fp32

F32 = mybir.dt.float32
BF16 = mybir.dt.bfloat16
MULT = mybir.AluOpType.mult
ADD = mybir.AluOpType.add
SUB = mybir.AluOpType.subtract


def _build_nc():
    nc = bacc.Bacc()
    x_d = nc.declare_dram_parameter("x", [S, D], F32, isOutput=False)
    whi_d = nc.declare_dram_parameter("w_hi", [D, EALL], BF16, isOutput=False)
    wlo_d = nc.declare_dram_parameter("w_lo", [D, EALL], BF16, isOutput=False)
    wohi_d = nc.declare_dram_parameter("wo_hi", [2 * P, D], BF16, isOutput=False)
    wolo_d = nc.declare_dram_parameter("wo_lo", [2 * P, D], BF16, isOutput=False)
    cos_d = nc.declare_dram_parameter("cos_d", [S, HD], F32, isOutput=False)
    sin_d = nc.declare_dram_parameter("sin_s", [S, HD], F32, isOutput=False)
    wn_d = nc.declare_dram_parameter("wnorm", [EQK], F32, isOutput=False)
    lam_d = nc.declare_dram_parameter("lam", [1], F32, isOutput=False)
    y_d = nc.declare_dram_parameter("y", [S, D], F32, isOutput=True)

    Tanh = mybir.ActivationFunctionType.Tanh
    Exp = mybir.ActivationFunctionType.Exp

    with tile.TileContext(nc) as tc:
        with (
            tc.tile_pool(name="singles", bufs=1) as singles,
            tc.tile_pool(name="persist", bufs=1) as persist,
            tc.tile_pool(name="work", bufs=2) as work,
            tc.tile_pool(name="tpool", bufs=3) as tpool,
            tc.tile_pool(name="atpool", bufs=3) as atpool,
            tc.tile_pool(name="small", bufs=9) as small,
            tc.tile_pool(name="psA", bufs=2, space="PSUM") as psA,
            tc.tile_pool(name="psB", bufs=2, space="PSUM") as psB,
            tc.tile_pool(name="psAT", bufs=2, space="PSUM") as psAT,
        ):
            # ---- one-time setup ----
            w_hi = singles.tile([P, KT, EALL], BF16)
            nc.sync.dma_start(w_hi, whi_d.rearrange("(t p) e -> p t e", p=P))
            w_lo = singles.tile([P, KT, EALL], BF16)
            nc.sync.dma_start(w_lo, wlo_d.rearrange("(t p) e -> p t e", p=P))
            ident = singles.tile([P, P], F32)
            make_identity(nc, ident)
            cos_sb = singles.tile([P, NSB, HD], F32)
            nc.sync.dma_start(cos_sb, cos_d.rearrange("(n p) f -> p n f", p=P))
            sin_sb = singles.tile([P, NSB, HD], F32)
            nc.sync.dma_start(sin_sb, sin_d.rearrange("(n p) f -> p n f", p=P))
            wo_hi = singles.tile([P, 2, D], BF16)
            nc.sync.dma_start(wo_hi, wohi_d.rearrange("(t p) e -> p t e", p=P))
            wo_lo = singles.tile([P, 2, D], BF16)
            nc.sync.dma_start(wo_lo, wolo_d.rearrange("(t p) e -> p t e", p=P))

            def part_bcast(handle):
                ap = handle[:]
                return bass.AP(tensor=ap.tensor, offset=ap.offset, ap=[[0, P], *ap.ap])

            wn_sb = singles.tile([P, EQK], F32)
            nc.gpsimd.dma_start(wn_sb, part_bcast(wn_d))
            lam_sb = singles.tile([P, 1], F32)
            nc.gpsimd.dma_start(lam_sb, part_bcast(lam_d))

            # persistent per-core activation storage
            v_sb = [persist.tile([P, EV], F32, name=f"v{i}", tag=f"v{i}")
                    for i in range(NSB)]
            qThi = [persist.tile([P, G, P], BF16, name=f"qThi{i}", tag=f"qThi{i}")
                    for i in range(NSB)]
            qTlo = [persist.tile([P, G, P], BF16, name=f"qTlo{i}", tag=f"qTlo{i}")
                    for i in range(NSB)]
            kThi = [persist.tile([P, 512], BF16, name=f"kThi{i}", tag=f"kThi{i}")
                    for i in range(NSB // 4)]
            kTlo = [persist.tile([P, 512], BF16, name=f"kTlo{i}", tag=f"kTlo{i}")
                    for i in range(NSB // 4)]

            def bcast_groups(src2d, n):
                return bass.AP(
                    tensor=src2d.tensor,
                    offset=src2d.offset,
                    ap=[src2d.ap[0], [0, n], src2d.ap[-1]],
                )

            def hilo_evict(psrc, hi, lo):
                """psum fp32 -> bf16 hi + bf16 lo (exact split)"""
                nc.vector.tensor_copy(hi, psrc)
                nc.vector.scalar_tensor_tensor(
                    out=lo, in0=hi, scalar=-1.0, in1=psrc, op0=MULT, op1=ADD
                )

            def phase1(si):
                """projections + rmsnorm + rope + transposes for s-block si"""
                x_sb = work.tile([P, D], F32, tag="x")
                nc.scalar.dma_start(x_sb, x_d[si * P:(si + 1) * P, :])
                # transpose x block: [s,d] -> [d,s], split to bf16 hi/lo
                xThi = work.tile([P, KT, P], BF16, tag="xThi")
                xTlo = work.tile([P, KT, P], BF16, tag="xTlo")
                for half in range(2):
                    pt = psB.tile([P, 512], F32, tag="pose")
                    for t in range(4):
                        tt = 4 * half + t
                        nc.tensor.transpose(
                            pt[:, t * P:(t + 1) * P], x_sb[:, tt * P:(tt + 1) * P],
                            ident,
                        )
                    sl = slice(4 * half, 4 * half + 4)
                    hilo_evict(pt, xThi[:, sl, :], xTlo[:, sl, :])
                # projections via hi/lo bf16 3-matmul
                pp = psA.tile([P, 1024], F32, tag="A")
                for t in range(KT):
                    first, last = t == 0, t == KT - 1
                    for co, cw in ((0, 512), (512, 192)):
                        po = pp[:, co:co + cw]
                        wsl = slice(co, co + cw)
                        nc.tensor.matmul(
                            po, xThi[:, t, :], w_hi[:, t, wsl],
                            start=first, stop=False,
                        )
                        nc.tensor.matmul(
                            po, xThi[:, t, :], w_lo[:, t, wsl],
                            start=False, stop=False,
                        )
                        nc.tensor.matmul(
                            po, xTlo[:, t, :], w_hi[:, t, wsl],
                            start=False, stop=last,
                        )
                # v: plain eviction (fp32)
                nc.vector.tensor_copy(v_sb[si], pp[:, EQK:EALL])
                # rmsnorm stats
                qk0 = work.tile([P, EQK], F32, tag="qk0")
                nc.vector.tensor_copy(qk0, pp[:, 0:EQK])
                sq = work.tile([P, EQK], F32, tag="m1")
                nc.gpsimd.tensor_mul(sq, qk0, qk0)
                ssq = small.tile([P, NG], F32, tag="ssq")
                nc.vector.tensor_reduce(
                    ssq, sq.rearrange("p (g d) -> p g d", d=HD),
                    axis=mybir.AxisListType.X, op=ADD,
                )
                # a = mean_sq + eps; rinv = rsqrt(a) via Newton on DVE
                # (keeps ScalarE on the exp/tanh table only)
                aa = small.tile([P, NG], F32, tag="aa")
                nc.vector.tensor_scalar(
                    out=aa, in0=ssq, scalar1=1.0 / HD, scalar2=EPS,
                    op0=MULT, op1=ADD,
                )
                rinv = small.tile([P, NG], F32, tag="rinv")
                nc.vector.reciprocal(rinv, aa)
                nc.vector.tensor_scalar_min(rinv, rinv, 1.0)
                t_n = small.tile([P, NG], F32, tag="t_n")
                for _ in range(5):
                    nc.vector.tensor_mul(t_n, rinv, rinv)
                    nc.vector.tensor_mul(t_n, t_n, aa)
                    nc.vector.tensor_scalar(
                        out=t_n, in0=t_n, scalar1=-0.5, scalar2=1.5,
                        op0=MULT, op1=ADD,
                    )
                    nc.vector.tensor_mul(rinv, rinv, t_n)
                # apply 1/rms and norm weight
                qk = work.tile([P, EQK], F32, tag="qk")
                for g in range(NG):
                    sl = slice(g * HD, (g + 1) * HD)
                    nc.vector.scalar_tensor_tensor(
                        out=qk[:, sl], in0=qk0[:, sl], scalar=rinv[:, g:g + 1],
                        in1=wn_sb[:, sl], op0=MULT, op1=MULT,
                    )
                # rope: out = qk * cos_dup + swap(qk) * sin_sign
                qkv = qk.rearrange("p (n two) -> p n two", two=2)
                xr = work.tile([P, EQK], F32, tag="qk0")
                xrv = xr.rearrange("p (n two) -> p n two", two=2)
                nc.gpsimd.tensor_copy(xrv[:, :, 0:1], qkv[:, :, 1:2])
                nc.gpsimd.tensor_copy(xrv[:, :, 1:2], qkv[:, :, 0:1])
                cosb = bcast_groups(cos_sb[:, si, :], NG)
                sinb = bcast_groups(sin_sb[:, si, :], NG)
                m1 = work.tile([P, EQK], F32, tag="m1")
                nc.gpsimd.tensor_mul(m1, qk, cosb)
                nc.vector.tensor_mul(xr, xr, sinb)
                nc.vector.tensor_add(qk, m1, xr)
                # transpose q heads and k to [dim, s]; split bf16 hi/lo
                pq = psB.tile([P, 512], F32, tag="pose")
                for h in range(G):
                    nc.tensor.transpose(
                        pq[:, h * P:(h + 1) * P], qk[:, h * P:(h + 1) * P], ident
                    )
                hilo_evict(pq, qThi[si], qTlo[si])
                pk = psB.tile([P, 512], F32, tag="pose")
                nc.tensor.transpose(pk[:, 0:P], qk[:, EQ:EQK], ident)
                ksl = slice((si % 4) * P, (si % 4 + 1) * P)
                hilo_evict(pk[:, 0:P], kThi[si // 4][:, ksl], kTlo[si // 4][:, ksl])

            def attention(qb):
                """attention + O-projection for q-block qb (all 4 heads)"""
                nkb = qb + 1
                L = nkb * P
                oThi = [small.tile([P, P], BF16, name=f"oThi{qb}_{hp}", tag=f"oThi{hp}")
                        for hp in range(2)]
                oTlo = [small.tile([P, P], BF16, name=f"oTlo{qb}_{hp}", tag=f"oTlo{hp}")
                        for hp in range(2)]
                for hp in range(2):
                    at_sb = []
                    for hh in range(2):
                        h = 2 * hp + hh
                        t12 = tpool.tile([P, 2, S], F32, tag="t")
                        t1 = t12[:, 0, :]
                        t2 = t12[:, 1, :]
                        # scores: hi/lo 3-matmul, s1/s2 row-paired
                        for kc in range(0, L, 512):
                            w = min(512, L - kc)
                            sc = psA.tile([P, 1024], F32, tag="A")
                            ci = kc // 512
                            for off, qh in ((0, 0), (512, HD)):
                                khi = kThi[ci][qh:qh + HD, 0:w]
                                klo = kTlo[ci][qh:qh + HD, 0:w]
                                qhi = qThi[qb][qh:qh + HD, h, :]
                                qlo = qTlo[qb][qh:qh + HD, h, :]
                                po = sc[:, off:off + w]
                                nc.tensor.matmul(po, qhi, khi, start=True, stop=False)
                                nc.tensor.matmul(po, qhi, klo, start=False, stop=False)
                                nc.tensor.matmul(po, qlo, khi, start=False, stop=True)
                            # one tanh over both score halves
                            nc.scalar.activation(
                                t12[:, :, kc:kc + w],
                                sc.rearrange("p (m c) -> p m c", m=2)[:, :, 0:w],
                                Tanh, scale=SCALE / CAP,
                            )
                        # causal mask on diagonal block (keep where row >= col)
                        for t in (t1, t2):
                            nc.gpsimd.affine_select(
                                out=t[:, qb * P:L], in_=t[:, qb * P:L],
                                compare_op=mybir.AluOpType.is_ge, fill=MASK_FILL,
                                base=0, pattern=[[-1, P]], channel_multiplier=1,
                            )
                        # exp in place with row-sum accumulation
                        r1 = small.tile([P, 1], F32, tag="r")
                        r2 = small.tile([P, 1], F32, tag="r")
                        nc.scalar.activation(
                            t1[:, 0:L], t1[:, 0:L], Exp, scale=CAP, accum_out=r1
                        )
                        nc.scalar.activation(
                            t2[:, 0:L], t2[:, 0:L], Exp, scale=CAP, accum_out=r2
                        )
                        r1i = small.tile([P, 1], F32, tag="r")
                        nc.vector.reciprocal(r1i, r1)
                        r2i = small.tile([P, 1], F32, tag="r")
                        nc.vector.reciprocal(r2i, r2)
                        nr2i = small.tile([P, 1], F32, tag="r")
                        nc.vector.tensor_scalar(
                            out=nr2i, in0=r2i, scalar1=lam_sb[:, 0:1], scalar2=-1.0,
                            op0=MULT, op1=MULT,
                        )
                        # attn_diff^T via two accumulated fp32 transposes;
                        # e1/e2 normalized per 512-chunk right before their
                        # transposes so PE starts ~1 chunk after r1 is ready
                        a_sb = atpool.tile([P, S], F32, tag="at")
                        for kc in range(0, L, 512):
                            w = min(512, L - kc)
                            nc.vector.tensor_scalar_mul(
                                t1[:, kc:kc + w], t1[:, kc:kc + w], r1i[:, 0:1]
                            )
                            nc.vector.tensor_scalar_mul(
                                t2[:, kc:kc + w], t2[:, kc:kc + w], nr2i[:, 0:1]
                            )
                            at4 = psAT.tile([P, 512], F32, tag="atpo")
                            # one bank-clearing start, then per-element
                            # overwrite (e1, has_written unset) / accumulate
                            # (e2, over e1's bits)
                            for kk in range(0, w, P):
                                sl = slice(kc + kk, kc + kk + P)
                                nc.tensor.matmul(
                                    at4[:, kk:kk + P], t1[:, sl], ident,
                                    is_transpose=True, start=(kk == 0), stop=False,
                                )
                            for kk in range(0, w, P):
                                sl = slice(kc + kk, kc + kk + P)
                                nc.tensor.matmul(
                                    at4[:, kk:kk + P], t2[:, sl], ident,
                                    is_transpose=True, start=False,
                                    stop=(kk + P >= w),
                                )
                            nc.vector.tensor_copy(a_sb[:, kc:kc + w], at4[:, 0:w])
                        at_sb.append(a_sb)
                    # AV: head pair via column tiling
                    po = psAT.tile([P, P], F32, tag="atpo", padded_shape=[P, 512])
                    for kb in range(nkb):
                        sl = slice(kb * P, (kb + 1) * P)
                        nc.tensor.matmul(
                            po[0:HD, :], v_sb[kb], at_sb[0][:, sl],
                            start=(kb == 0), stop=(kb == nkb - 1),
                            tile_position=(0, 0),
                        )
                        nc.tensor.matmul(
                            po[HD:P, :], v_sb[kb], at_sb[1][:, sl],
                            start=(kb == 0), stop=(kb == nkb - 1),
                            tile_position=(0, 64),
                        )
                    hilo_evict(po, oThi[hp], oTlo[hp])
                # O-projection (hi/lo bf16 3-matmul)
                y_sb = work.tile([P, D], F32, tag="y")
                for ch in range(2):
                    py = psB.tile([P, 512], F32, tag="pose")
                    sl = slice(ch * 512, (ch + 1) * 512)
                    nc.tensor.matmul(py, oThi[0], wo_hi[:, 0, sl],
                                     start=True, stop=False)
                    nc.tensor.matmul(py, oThi[0], wo_lo[:, 0, sl],
                                     start=False, stop=False)
                    nc.tensor.matmul(py, oTlo[0], wo_hi[:, 0, sl],
                                     start=False, stop=False)
                    nc.tensor.matmul(py, oThi[1], wo_hi[:, 1, sl],
                                     start=False, stop=False)
                    nc.tensor.matmul(py, oThi[1], wo_lo[:, 1, sl],
                                     start=False, stop=False)
                    nc.tensor.matmul(py, oTlo[1], wo_hi[:, 1, sl],
                                     start=False, stop=True)
                    nc.vector.tensor_copy(y_sb[:, sl], py)
                nc.sync.dma_start(y_d[qb * P:(qb + 1) * P, :], y_sb)

            # software pipeline: keep phase1 two s-blocks ahead so the PE
            # always has attention matmuls available while DVE/GPSIMD run
            # the norm/rope chain of upcoming blocks
            import os as _os
            LOOKAHEAD = int(_os.environ.get("K_LOOKAHEAD", "2"))
            for si in range(min(LOOKAHEAD, NSB)):
                phase1(si)
            for si in range(NSB):
                attention(si)
                if si + LOOKAHEAD < NSB:
                    phase1(si + LOOKAHEAD)

    nc.finalize()
    return nc


_NC = None


def _get_nc():
    global _NC
    if _NC is None:
        _NC = _build_nc()
    return _NC


def _hilo(a):
    hi = a.astype(ml_dtypes.bfloat16)
    lo = (a - hi.astype(np.float32)).astype(ml_dtypes.bfloat16)
    return hi, lo


def kernel(x, rope_freqs, wq, wk, wv, wo, q_norm_w, k_norm_w, diff_lambda):
    x = np.asarray(x, dtype=np.float32)
    rope_freqs = np.asarray(rope_freqs, dtype=np.float32)
    wq, wk, wv, wo = (np.asarray(a, dtype=np.float32) for a in (wq, wk, wv, wo))
    q_norm_w = np.asarray(q_norm_w, dtype=np.float32)
    k_norm_w = np.asarray(k_norm_w, dtype=np.float32)
    diff_lambda = np.asarray(diff_lambda, dtype=np.float32)

    cos = np.repeat(rope_freqs[:, :, 0], 2, axis=1).astype(np.float32)
    sin = np.repeat(rope_freqs[:, :, 1], 2, axis=1).astype(np.float32)
    sin_s = sin.copy()
    sin_s[:, 0::2] *= -1.0
    wnorm = np.concatenate(
        [np.tile(q_norm_w, 2 * G), np.tile(k_norm_w, 2)]
    ).astype(np.float32)

    in_maps = []
    for c in range(8):
        b, j = divmod(c, KV)
        w_all_t = np.ascontiguousarray(
            np.concatenate(
                [
                    wq[EQ * j:EQ * (j + 1), :],
                    wk[EK * j:EK * (j + 1), :],
                    wv[EV * j:EV * (j + 1), :],
                ],
                axis=0,
            ).T
        )
        wo_t = np.ascontiguousarray(wo[:, 2 * P * j:2 * P * (j + 1)].T)
        w_hi, w_lo = _hilo(w_all_t)
        wo_hi, wo_lo = _hilo(wo_t)
        in_maps.append(
            {
                "x": np.ascontiguousarray(x[b]),
                "w_hi": w_hi,
                "w_lo": w_lo,
                "wo_hi": wo_hi,
                "wo_lo": wo_lo,
                "cos_d": cos,
                "sin_s": sin_s,
                "wnorm": wnorm,
                "lam": diff_lambda.reshape(1),
            }
        )

    nc = _get_nc()
    trace = os.environ.get("KERNEL_TRACE") == "1"
    res = run_bass_kernel_spmd(nc, in_maps, core_ids=list(range(8)), trace=trace)
    if trace and res.exec_time_ns is not None:
        print(f"HW exec time: {res.exec_time_ns} ns")

    out = np.zeros((B, S, D), dtype=np.float32)
    for c in range(8):
        b = c // KV
        out[b] += res.results[c]["y"]
    return out



# revision 9
# speedup vs baseline: 1.2477x; 1.2477x over previous
"""Trainium2 Bass kernel for nn_CausalAttention (diff-attention with QK-norm,
RoPE, GQA, tanh soft-cap, causal softmax).

Sharding: 8 cores = (batch b in {0,1}) x (kv-group j in {0..3}).
Each core handles one batch element and the 4 query heads of one kv head.

Perf strategy (vs the hi/lo-bf16 baseline):
  - all big GEMMs (x-transpose, projections, scores, O-proj) run in fp32r
    (1 cyc/row for moving dim >= 256, ~1.5e-4 rel err) instead of 3x
    bf16 hi/lo matmuls
  - RMSNorm rinv is applied pre-rope on DVE; q_norm_w/k_norm_w are folded
    into the rope cos/sin tables host-side (w varies per dim inside a rope
    pair, so it must ride the tables, not a post-rope scale)
  - softmax normalization (1/r1, -lambda/r2) is folded into the
    attn-transpose step: A^T = e1.T @ diag(1/r1) + e2.T @ diag(-lam/r2)
    as two accumulated regular bf16 matmuls (1 cyc/row); exp outputs bf16
  - AV runs bf16 (V eviction to bf16), O-proj fp32r
  - ScalarE does only tanh + exp (one act table, loaded once); rsqrt stays
    a DVE Newton chain so no table reloads ever happen
"""

import os
import sys

import numpy as np

if "/opt/trn_rl_repo" not in sys.path:
    sys.path.insert(0, "/opt/trn_rl_repo")

import concourse.bass as bass
import concourse.mybir as mybir
import concourse.tile as tile
from concourse import bacc
from concourse.bass_utils import run_bass_kernel_spmd
from concourse.masks import make_identity

B, S, D = 2, 2048, 1024
H, KV, HD = 16, 4, 64
G = H // KV          # q heads per kv head (= heads per core)
CAP = 50.0
EPS = 1e-6
SCALE = 1.0 / 8.0    # 1/sqrt(HD)
P = 128
NSB = S // P         # 16 s-blocks
EQ = G * 2 * HD      # 512 q-projection cols per core
EK = 2 * HD          # 128 k-projection cols per core
EV = HD              # 64  v-projection cols per core
EQK = EQ + EK        # 640 cols needing norm+rope
EALL = EQ + EK + EV  # 704 projection cols per core
EPAD = 768           # proj width padded to psum-bank-aligned 512+256 groups
NG = EQK // HD       # 10 rmsnorm groups
KT = D // P          # 8 contraction tiles
MASK_FILL = -1.0e4   # exp(50 * -1e4) == 0 in fp32

F32 = mybir.dt.float32
F32R = mybir.dt.float32r
BF16 = mybir.dt.bfloat16
MULT = mybir.AluOpType.mult
ADD = mybir.AluOpType.add
Tanh = mybir.ActivationFunctionType.Tanh
Exp = mybir.ActivationFunctionType.Exp


def _build_nc():
    nc = bacc.Bacc()
    x_d = nc.declare_dram_parameter("x", [S, D], F32R, isOutput=False)
    w_d = nc.declare_dram_parameter("w", [D, EPAD], F32R, isOutput=False)
    wo_d = nc.declare_dram_parameter("wo", [2 * P, D], F32R, isOutput=False)
    cosq_d = nc.declare_dram_parameter("cosq", [S, HD], F32, isOutput=False)
    sinq_d = nc.declare_dram_parameter("sinq", [S, HD], F32, isOutput=False)
    cosk_d = nc.declare_dram_parameter("cosk", [S, HD], F32, isOutput=False)
    sink_d = nc.declare_dram_parameter("sink", [S, HD], F32, isOutput=False)
    lam_d = nc.declare_dram_parameter("lam", [1], F32, isOutput=False)
    y_d = nc.declare_dram_parameter("y", [S, D], F32, isOutput=True)

    with tile.TileContext(nc) as tc:
        with (
            tc.tile_pool(name="singles", bufs=1) as singles,
            tc.tile_pool(name="persist", bufs=1) as persist,
            tc.tile_pool(name="work", bufs=2) as work,
            tc.tile_pool(name="t12p", bufs=1) as t12p,
            tc.tile_pool(name="atp", bufs=2) as atp,
            tc.tile_pool(name="small", bufs=6) as small,
            tc.tile_pool(name="diagp", bufs=2) as diagp,
            tc.tile_pool(name="otp", bufs=2) as otp,
            tc.tile_pool(name="psA", bufs=2, space="PSUM") as psA,
            tc.tile_pool(name="psAT", bufs=2, space="PSUM") as psAT,
            tc.tile_pool(name="psB", bufs=1, space="PSUM") as psB,
            tc.tile_pool(name="psC", bufs=1, space="PSUM") as psC,
        ):
            # ---- one-time setup ----
            w_sb = singles.tile([P, KT, EPAD], F32R)
            nc.sync.dma_start(w_sb, w_d.rearrange("(t p) e -> p t e", p=P))
            wo_sb = singles.tile([P, 2, D], F32R)
            nc.sync.dma_start(wo_sb, wo_d.rearrange("(t p) e -> p t e", p=P))
            cosq = singles.tile([P, NSB, HD], F32)
            nc.scalar.dma_start(cosq, cosq_d.rearrange("(n p) f -> p n f", p=P))
            sinq = singles.tile([P, NSB, HD], F32)
            nc.scalar.dma_start(sinq, sinq_d.rearrange("(n p) f -> p n f", p=P))
            cosk = singles.tile([P, NSB, HD], F32)
            nc.gpsimd.dma_start(cosk, cosk_d.rearrange("(n p) f -> p n f", p=P))
            sink = singles.tile([P, NSB, HD], F32)
            nc.gpsimd.dma_start(sink, sink_d.rearrange("(n p) f -> p n f", p=P))

            ident_f = singles.tile([P, P], F32)
            make_identity(nc, ident_f)
            ident_r = singles.tile([P, P], F32R)
            nc.vector.tensor_copy(ident_r, ident_f)
            ident_bf = singles.tile([P, P], BF16)
            nc.gpsimd.tensor_copy(ident_bf, ident_f)

            def part_bcast(handle):
                ap = handle[:]
                return bass.AP(tensor=ap.tensor, offset=ap.offset,
                               ap=[[0, P], *ap.ap])

            lam_sb = singles.tile([P, 1], F32)
            nc.gpsimd.dma_start(lam_sb, part_bcast(lam_d))

            # persistent per-core activation storage
            v_sb = [persist.tile([P, EV], BF16, name=f"v{i}", tag=f"v{i}")
                    for i in range(NSB)]
            qT = [persist.tile([P, G, P], F32R, name=f"qT{i}", tag=f"qT{i}")
                  for i in range(NSB)]
            kTg = [persist.tile([P, 4 * P], F32R, name=f"kT{i}", tag=f"kT{i}")
                   for i in range(NSB // 4)]

            def bcast_groups(src2d, n):
                return bass.AP(
                    tensor=src2d.tensor,
                    offset=src2d.offset,
                    ap=[src2d.ap[0], [0, n], src2d.ap[-1]],
                )

            def phase1(si):
                """projections + rmsnorm + rope + transposes for s-block si"""
                x_sb = work.tile([P, D], F32R, tag="x")
                nc.sync.dma_start(x_sb, x_d[si * P:(si + 1) * P, :])
                # transpose x block: [s,d] -> [d,s] (fp32r, 1.5 cyc/row)
                xT = work.tile([P, KT, P], F32R, tag="xT")
                for half in range(2):
                    pt = psB.tile([P, 512], F32, tag="B")
                    ptr = pt[:].bitcast(F32R)
                    for t in range(4):
                        tt = 4 * half + t
                        nc.tensor.transpose(
                            ptr[:, t * P:(t + 1) * P],
                            x_sb[:, tt * P:(tt + 1) * P], ident_r,
                        )
                    nc.vector.tensor_copy(
                        xT[:, 4 * half:4 * half + 4, :].rearrange(
                            "p a b -> p (a b)"), pt)
                # projections: fp32r, two 352-wide psum groups
                pp = psA.tile([P, 1024], F32, tag="A")
                for co, cw in ((0, 512), (512, 256)):
                    for t in range(KT):
                        nc.tensor.matmul(
                            pp[:, co:co + cw], xT[:, t, :],
                            w_sb[:, t, co:co + cw],
                            start=(t == 0), stop=(t == KT - 1),
                        )
                # v: eviction to bf16
                nc.vector.tensor_copy(v_sb[si], pp[:, EQK:EALL])
                # rmsnorm stats on raw projections
                sq = work.tile([P, EQK], F32, tag="sq")
                nc.scalar.activation(
                    sq, pp[:, 0:EQK], mybir.ActivationFunctionType.Square)
                ssq = small.tile([P, NG], F32, tag="ssq")
                nc.vector.tensor_reduce(
                    ssq, sq.rearrange("p (g d) -> p g d", d=HD),
                    axis=mybir.AxisListType.X, op=ADD,
                )
                aa = small.tile([P, NG], F32, tag="aa")
                nc.gpsimd.tensor_scalar(
                    out=aa, in0=ssq, scalar1=1.0 / HD, scalar2=EPS,
                    op0=MULT, op1=ADD,
                )
                # rinv = rsqrt(aa) via Newton on DVE (keeps ScalarE on the
                # exp/tanh table only)
                rinv = small.tile([P, NG], F32, tag="rinv")
                nc.vector.reciprocal(rinv, aa)
                nc.vector.tensor_scalar_min(rinv, rinv, 1.0)
                t_n = small.tile([P, NG], F32, tag="t_n")
                for _ in range(5):
                    nc.vector.tensor_mul(t_n, rinv, rinv)
                    nc.vector.tensor_mul(t_n, t_n, aa)
                    nc.vector.tensor_scalar(
                        out=t_n, in0=t_n, scalar1=-0.5, scalar2=1.5,
                        op0=MULT, op1=ADD,
                    )
                    nc.vector.tensor_mul(rinv, rinv, t_n)
                # apply 1/rms (pre-rope; norm weight rides the rope tables)
                qkn = work.tile([P, EQK], F32, tag="qkn")
                for g in range(NG):
                    sl = slice(g * HD, (g + 1) * HD)
                    nc.vector.tensor_scalar_mul(
                        qkn[:, sl], pp[:, sl], rinv[:, g:g + 1])
                # rope: out = qkn * cosw_dup + swap(qkn) * sinw_signed
                m1 = work.tile([P, EQK], F32, tag="sq")
                nc.gpsimd.tensor_mul(
                    m1[:, 0:EQ], qkn[:, 0:EQ],
                    bcast_groups(cosq[:, si, :], EQ // HD))
                nc.gpsimd.tensor_mul(
                    m1[:, EQ:EQK], qkn[:, EQ:EQK],
                    bcast_groups(cosk[:, si, :], EK // HD))
                xr = work.tile([P, EQK], F32, tag="xr")
                xrv = xr.rearrange("p (n two) -> p n two", two=2)
                qknv = qkn.rearrange("p (n two) -> p n two", two=2)
                nc.gpsimd.tensor_copy(xrv[:, :, 0:1], qknv[:, :, 1:2])
                nc.gpsimd.tensor_copy(xrv[:, :, 1:2], qknv[:, :, 0:1])
                nc.gpsimd.tensor_mul(
                    xr[:, 0:EQ], xr[:, 0:EQ],
                    bcast_groups(sinq[:, si, :], EQ // HD))
                nc.gpsimd.tensor_mul(
                    xr[:, EQ:EQK], xr[:, EQ:EQK],
                    bcast_groups(sink[:, si, :], EK // HD))
                qkr = work.tile([P, EQK], F32R, tag="qkr")
                nc.gpsimd.tensor_add(qkr, m1, xr)
                # transpose q heads and k to [dim, s] (fp32r)
                pq = psB.tile([P, 512], F32, tag="B")
                pqr = pq[:].bitcast(F32R)
                for h in range(G):
                    nc.tensor.transpose(
                        pqr[:, h * P:(h + 1) * P], qkr[:, h * P:(h + 1) * P],
                        ident_r,
                    )
                nc.vector.tensor_copy(qT[si][:].rearrange("p g s -> p (g s)"), pq)
                pk = psB.tile([P, 512], F32, tag="B")
                pkr = pk[:].bitcast(F32R)
                nc.tensor.transpose(pkr[:, 0:P], qkr[:, EQ:EQK], ident_r)
                ksl = slice((si % 4) * P, (si % 4 + 1) * P)
                nc.vector.tensor_copy(kTg[si // 4][:, ksl], pk[:, 0:P])

            def scores_head(qb, h, t12):
                """scores + tanh + mask + exp for one head; returns e12,r1,r2"""
                L = (qb + 1) * P
                for kc in range(0, L, 512):
                    w = min(512, L - kc)
                    sc = psA.tile([P, 1024], F32, tag="A")
                    sc2 = sc.rearrange("p (m c) -> p m c", m=2)
                    ci = kc // 512
                    for br in range(2):
                        qh = br * HD
                        nc.tensor.matmul(
                            sc2[:, br, 0:w],
                            qT[qb][qh:qh + HD, h, :],
                            kTg[ci][qh:qh + HD, 0:w],
                            start=True, stop=True,
                        )
                    nc.scalar.activation(
                        t12[:, :, kc:kc + w], sc2[:, :, 0:w],
                        Tanh, scale=SCALE / CAP,
                    )
                # causal mask on diagonal block (keep where row >= col)
                for br in range(2):
                    nc.gpsimd.affine_select(
                        out=t12[:, br, qb * P:L], in_=t12[:, br, qb * P:L],
                        compare_op=mybir.AluOpType.is_ge, fill=MASK_FILL,
                        base=0, pattern=[[-1, P]], channel_multiplier=1,
                    )
                e12 = work.tile([P, 2, S], BF16, tag="e12")
                r1 = small.tile([P, 1], F32, tag="r")
                r2 = small.tile([P, 1], F32, tag="r")
                nc.scalar.activation(
                    e12[:, 0, 0:L], t12[:, 0, 0:L], Exp, scale=CAP,
                    accum_out=r1)
                nc.scalar.activation(
                    e12[:, 1, 0:L], t12[:, 1, 0:L], Exp, scale=CAP,
                    accum_out=r2)
                return e12, r1, r2

            def attn_transpose(qb, h, e12, r1, r2, at_sb):
                """normalized diff attention, transposed:
                at = e1.T @ diag(1/r1) + e2.T @ diag(-lam/r2), bf16"""
                L = (qb + 1) * P
                r1i = small.tile([P, 1], F32, tag="r")
                nc.vector.reciprocal(r1i, r1)
                r2i = small.tile([P, 1], F32, tag="r")
                nc.vector.reciprocal(r2i, r2)
                nr2i = small.tile([P, 1], F32, tag="r")
                nc.vector.tensor_scalar(
                    out=nr2i, in0=r2i, scalar1=lam_sb[:, 0:1], scalar2=-1.0,
                    op0=MULT, op1=MULT,
                )
                diag1 = diagp.tile([P, P], BF16, tag="diag1")
                nc.gpsimd.tensor_scalar_mul(diag1, ident_bf, r1i[:, 0:1])
                diag2 = diagp.tile([P, P], BF16, tag="diag2")
                nc.gpsimd.tensor_scalar_mul(diag2, ident_bf, nr2i[:, 0:1])
                for kc in range(0, L, 512):
                    w = min(512, L - kc)
                    at4 = psAT.tile([P, 512], F32, tag="AT")
                    for kk in range(0, w, P):
                        sl = slice(kc + kk, kc + kk + P)
                        nc.tensor.matmul(
                            at4[:, kk:kk + P], e12[:, 0, sl], diag1,
                            start=(kk == 0), stop=False,
                        )
                    for kk in range(0, w, P):
                        sl = slice(kc + kk, kc + kk + P)
                        nc.tensor.matmul(
                            at4[:, kk:kk + P], e12[:, 1, sl], diag2,
                            start=False, stop=(kk + P >= w),
                        )
                    nc.vector.tensor_copy(at_sb[:, kc:kc + w], at4[:, 0:w])

            def attention(qb):
                nkb = qb + 1
                oT = [otp.tile([P, P], F32R, name=f"oT{qb}_{hp}",
                               tag=f"oT{hp}") for hp in range(2)]
                ats = [None, None]
                es = {}

                def do_post(h):
                    e12, r1, r2 = es.pop(h)
                    at_sb = atp.tile([P, S], BF16, tag=f"at{h % 2}",
                                     name=f"at_{qb}_{h}")
                    attn_transpose(qb, h, e12, r1, r2, at_sb)
                    ats[h % 2] = at_sb
                    if h % 2 == 1:
                        hp = h // 2
                        po = psC.tile([P, P], F32, tag="C", name=f"po{qb}{hp}")
                        for kb in range(nkb):
                            sl = slice(kb * P, (kb + 1) * P)
                            nc.tensor.matmul(
                                po[0:HD, :], v_sb[kb], ats[0][:, sl],
                                start=(kb == 0), stop=(kb == nkb - 1),
                                tile_position=(0, 0),
                            )
                            nc.tensor.matmul(
                                po[HD:P, :], v_sb[kb], ats[1][:, sl],
                                start=(kb == 0), stop=(kb == nkb - 1),
                                tile_position=(0, 64),
                            )
                        nc.vector.tensor_copy(oT[hp], po)

                # issue scores(h+1) on the PE before at4(h): the PE never
                # waits on head h's exp->recip->diag chain
                for h in range(G):
                    t12 = t12p.tile([P, 2, S], F32, tag=f"t12_{h % 2}",
                                    name=f"t12_{qb}_{h}")
                    es[h] = scores_head(qb, h, t12)
                    if h >= 1:
                        do_post(h - 1)
                do_post(G - 1)
                # O-projection (fp32r)
                y_sb = work.tile([P, D], F32, tag="y")
                for ch in range(2):
                    py = psB.tile([P, 512], F32, tag="B")
                    sl = slice(ch * 512, (ch + 1) * 512)
                    nc.tensor.matmul(py, oT[0],
                                     wo_sb[:, 0, sl], start=True, stop=False)
                    nc.tensor.matmul(py, oT[1],
                                     wo_sb[:, 1, sl], start=False, stop=True)
                    nc.vector.tensor_copy(y_sb[:, sl], py)
                nc.sync.dma_start(y_d[qb * P:(qb + 1) * P, :], y_sb)

            # software pipeline: keep phase1 two s-blocks ahead
            LOOKAHEAD = int(os.environ.get("K_LOOKAHEAD", "2"))
            for si in range(min(LOOKAHEAD, NSB)):
                phase1(si)
            for qb in range(NSB):
                attention(qb)
                if qb + LOOKAHEAD < NSB:
                    phase1(qb + LOOKAHEAD)

    nc.finalize()
    return nc


_NC = None


def _get_nc():
    global _NC
    if _NC is None:
        _NC = _build_nc()
    return _NC


def kernel(x, rope_freqs, wq, wk, wv, wo, q_norm_w, k_norm_w, diff_lambda):
    x = np.asarray(x, dtype=np.float32)
    rope_freqs = np.asarray(rope_freqs, dtype=np.float32)
    wq, wk, wv, wo = (np.asarray(a, dtype=np.float32) for a in (wq, wk, wv, wo))
    q_norm_w = np.asarray(q_norm_w, dtype=np.float32)
    k_norm_w = np.asarray(k_norm_w, dtype=np.float32)
    diff_lambda = np.asarray(diff_lambda, dtype=np.float32)

    cos = np.repeat(rope_freqs[:, :, 0], 2, axis=1).astype(np.float32)
    sin = np.repeat(rope_freqs[:, :, 1], 2, axis=1).astype(np.float32)
    sin_s = sin.copy()
    sin_s[:, 0::2] *= -1.0
    # norm weights folded into the rope tables:
    #   out0 = w0 x0 c - w1 x1 s = x0*(c w0) + swap(x)0*(sin_s0 * w1)
    #   out1 = w1 x1 c + w0 x0 s = x1*(c w1) + swap(x)1*(sin_s1 * w0)
    qw = np.asarray(q_norm_w)
    kw = np.asarray(k_norm_w)
    qw_sw = qw.reshape(-1, 2)[:, ::-1].reshape(-1)
    kw_sw = kw.reshape(-1, 2)[:, ::-1].reshape(-1)
    cosq = (cos * qw[None, :]).astype(np.float32)
    sinq = (sin_s * qw_sw[None, :]).astype(np.float32)
    cosk = (cos * kw[None, :]).astype(np.float32)
    sink = (sin_s * kw_sw[None, :]).astype(np.float32)

    in_maps = []
    for c in range(8):
        b, j = divmod(c, KV)
        w_all_t = np.zeros((D, EPAD), dtype=np.float32)
        w_all_t[:, 0:EALL] = np.concatenate(
            [
                wq[EQ * j:EQ * (j + 1), :],
                wk[EK * j:EK * (j + 1), :],
                wv[EV * j:EV * (j + 1), :],
            ],
            axis=0,
        ).T
        wo_t = np.ascontiguousarray(wo[:, 2 * P * j:2 * P * (j + 1)].T)
        in_maps.append(
            {
                "x": np.ascontiguousarray(x[b]),
                "w": w_all_t,
                "wo": wo_t,
                "cosq": cosq,
                "sinq": sinq,
                "cosk": cosk,
                "sink": sink,
                "lam": diff_lambda.reshape(1),
            }
        )

    nc = _get_nc()
    trace = os.environ.get("KERNEL_TRACE") == "1"
    res = run_bass_kernel_spmd(nc, in_maps, core_ids=list(range(8)), trace=trace)
    if trace and res.exec_time_ns is not None:
        print(f"HW exec time: {res.exec_time_ns} ns")

    out = np.zeros((B, S, D), dtype=np.float32)
    for c in range(8):
        b = c // KV
        out[b] += res.results[c]["y"]
    return out


# revision 14
# speedup vs baseline: 1.5502x; 1.2424x over previous
"""Trainium2 Bass kernel for nn_CausalAttention (diff-attention with QK-norm,
RoPE, GQA, tanh soft-cap, causal softmax).

Sharding: 8 cores = (batch b in {0,1}) x (kv-group j in {0..3}).
Each core handles one batch element and the 4 query heads of one kv head.

Perf strategy (vs the hi/lo-bf16 baseline):
  - all big GEMMs (x-transpose, projections, scores, O-proj) run in fp32r
    (1 cyc/row for moving dim >= 256, ~1.5e-4 rel err) instead of 3x
    bf16 hi/lo matmuls
  - RMSNorm rinv is applied pre-rope on DVE; q_norm_w/k_norm_w are folded
    into the rope cos/sin tables host-side (w varies per dim inside a rope
    pair, so it must ride the tables, not a post-rope scale)
  - softmax normalization (1/r1, -lambda/r2) is folded into the
    attn-transpose step: A^T = e1.T @ diag(1/r1) + e2.T @ diag(-lam/r2)
    as two accumulated regular bf16 matmuls (1 cyc/row); exp outputs bf16
  - AV runs bf16 (V eviction to bf16), O-proj fp32r
  - ScalarE does only tanh + exp (one act table, loaded once); rsqrt stays
    a DVE Newton chain so no table reloads ever happen
"""

import os
import sys

import numpy as np

if "/opt/trn_rl_repo" not in sys.path:
    sys.path.insert(0, "/opt/trn_rl_repo")

import concourse.bass as bass
import concourse.mybir as mybir
import concourse.tile as tile
from concourse import bacc
from concourse.bass_utils import run_bass_kernel_spmd
from concourse.masks import make_identity

B, S, D = 2, 2048, 1024
H, KV, HD = 16, 4, 64
G = H // KV          # q heads per kv head (= heads per core)
CAP = 50.0
EPS = 1e-6
SCALE = 1.0 / 8.0    # 1/sqrt(HD)
P = 128
NSB = S // P         # 16 s-blocks
EQ = G * 2 * HD      # 512 q-projection cols per core
EK = 2 * HD          # 128 k-projection cols per core
EV = HD              # 64  v-projection cols per core
EQK = EQ + EK        # 640 cols needing norm+rope
EALL = EQ + EK + EV  # 704 projection cols per core
EPAD = 768           # proj width padded to psum-bank-aligned 512+256 groups
NG = EQK // HD       # 10 rmsnorm groups
KT = D // P          # 8 contraction tiles
MASK_FILL = -1.0e4   # exp(50 * -1e4) == 0 in fp32

F32 = mybir.dt.float32
F32R = mybir.dt.float32r
BF16 = mybir.dt.bfloat16
MULT = mybir.AluOpType.mult
ADD = mybir.AluOpType.add
Tanh = mybir.ActivationFunctionType.Tanh
Exp = mybir.ActivationFunctionType.Exp


def _build_nc():
    nc = bacc.Bacc()
    x_d = nc.declare_dram_parameter("x", [S, D], F32R, isOutput=False)
    w_d = nc.declare_dram_parameter("w", [D, EPAD], F32R, isOutput=False)
    wo_d = nc.declare_dram_parameter("wo", [2 * P, D], F32R, isOutput=False)
    cosq_d = nc.declare_dram_parameter("cosq", [S, HD], F32, isOutput=False)
    sinq_d = nc.declare_dram_parameter("sinq", [S, HD], F32, isOutput=False)
    cosk_d = nc.declare_dram_parameter("cosk", [S, HD], F32, isOutput=False)
    sink_d = nc.declare_dram_parameter("sink", [S, HD], F32, isOutput=False)
    lam_d = nc.declare_dram_parameter("lam", [1], F32, isOutput=False)
    y_d = nc.declare_dram_parameter("y", [S, D], F32, isOutput=True)

    with tile.TileContext(nc) as tc:
        with (
            tc.tile_pool(name="singles", bufs=1) as singles,
            tc.tile_pool(name="persist", bufs=1) as persist,
            tc.tile_pool(name="work", bufs=2) as work,
            tc.tile_pool(name="t12p", bufs=1) as t12p,
            tc.tile_pool(name="atp", bufs=2) as atp,
            tc.tile_pool(name="small", bufs=6) as small,
            tc.tile_pool(name="diagp", bufs=2) as diagp,
            tc.tile_pool(name="otp", bufs=2) as otp,
            tc.tile_pool(name="psA", bufs=2, space="PSUM") as psA,
            tc.tile_pool(name="psAT", bufs=2, space="PSUM") as psAT,
            tc.tile_pool(name="psB", bufs=1, space="PSUM") as psB,
            tc.tile_pool(name="psC", bufs=1, space="PSUM") as psC,
        ):
            # ---- one-time setup ----
            w_t = []
            for t in range(KT):
                wt = singles.tile([P, EPAD], F32R, name=f"w{t}", tag=f"w{t}")
                eng = nc.sync if t % 2 == 0 else nc.scalar
                eng.dma_start(wt, w_d[t * P:(t + 1) * P, :])
                w_t.append(wt)
            wo_sb = singles.tile([P, 2, D], F32R)
            nc.sync.dma_start(wo_sb, wo_d.rearrange("(t p) e -> p t e", p=P))
            cosq = singles.tile([P, NSB, HD], F32)
            nc.scalar.dma_start(cosq, cosq_d.rearrange("(n p) f -> p n f", p=P))
            sinq = singles.tile([P, NSB, HD], F32)
            nc.scalar.dma_start(sinq, sinq_d.rearrange("(n p) f -> p n f", p=P))
            cosk = singles.tile([P, NSB, HD], F32)
            nc.gpsimd.dma_start(cosk, cosk_d.rearrange("(n p) f -> p n f", p=P))
            sink = singles.tile([P, NSB, HD], F32)
            nc.gpsimd.dma_start(sink, sink_d.rearrange("(n p) f -> p n f", p=P))

            ident_f = singles.tile([P, P], F32)
            make_identity(nc, ident_f)
            ident_r = singles.tile([P, P], F32R)
            nc.vector.tensor_copy(ident_r, ident_f)
            ident_bf = singles.tile([P, P], BF16)
            nc.gpsimd.tensor_copy(ident_bf, ident_f)
            from concourse.masks import make_causal_mask
            maskm_f = singles.tile([P, P], F32)
            make_causal_mask(nc, maskm_f[:], mask_val=-1.0e8)
            maskm_r = singles.tile([P, P], F32R)
            nc.vector.tensor_copy(maskm_r, maskm_f)

            def part_bcast(handle):
                ap = handle[:]
                return bass.AP(tensor=ap.tensor, offset=ap.offset,
                               ap=[[0, P], *ap.ap])

            lam_sb = singles.tile([P, 1], F32)
            nc.gpsimd.dma_start(lam_sb, part_bcast(lam_d))

            # persistent per-core activation storage
            v_sb = [persist.tile([P, EV], BF16, name=f"v{i}", tag=f"v{i}")
                    for i in range(NSB)]
            qT = [persist.tile([P, G, P], F32R, name=f"qT{i}", tag=f"qT{i}")
                  for i in range(NSB)]
            kTg = [persist.tile([P, 4 * P], F32R, name=f"kT{i}", tag=f"kT{i}")
                   for i in range(NSB // 4)]

            def bcast_groups(src2d, n):
                return bass.AP(
                    tensor=src2d.tensor,
                    offset=src2d.offset,
                    ap=[src2d.ap[0], [0, n], src2d.ap[-1]],
                )

            def phase1(si):
                """projections + rmsnorm + rope + transposes for s-block si"""
                x_sb = work.tile([P, D], F32R, tag="x")
                nc.sync.dma_start(x_sb, x_d[si * P:(si + 1) * P, :])
                # transpose x block: [s,d] -> [d,s] (fp32r, 1.5 cyc/row)
                xT = work.tile([P, KT, P], F32R, tag="xT")
                for half in range(2):
                    pt = psB.tile([P, 512], F32, tag="B")
                    ptr = pt[:].bitcast(F32R)
                    for t in range(4):
                        tt = 4 * half + t
                        nc.tensor.transpose(
                            ptr[:, t * P:(t + 1) * P],
                            x_sb[:, tt * P:(tt + 1) * P], ident_r,
                        )
                    nc.vector.tensor_copy(
                        xT[:, 4 * half:4 * half + 4, :].rearrange(
                            "p a b -> p (a b)"), pt)
                # projections: fp32r, two 352-wide psum groups
                pp = psA.tile([P, 1024], F32, tag="A")
                for co, cw in ((0, 512), (512, 256)):
                    for t in range(KT):
                        nc.tensor.matmul(
                            pp[:, co:co + cw], xT[:, t, :],
                            w_t[t][:, co:co + cw],
                            start=(t == 0), stop=(t == KT - 1),
                        )
                # v: eviction to bf16
                nc.vector.tensor_copy(v_sb[si], pp[:, EQK:EALL])
                # rmsnorm stats on raw projections
                sq = work.tile([P, EQK], F32, tag="sq")
                nc.scalar.activation(
                    sq, pp[:, 0:EQK], mybir.ActivationFunctionType.Square)
                ssq = small.tile([P, NG], F32, tag="ssq")
                nc.vector.tensor_reduce(
                    ssq, sq.rearrange("p (g d) -> p g d", d=HD),
                    axis=mybir.AxisListType.X, op=ADD,
                )
                aa = small.tile([P, NG], F32, tag="aa")
                nc.gpsimd.tensor_scalar(
                    out=aa, in0=ssq, scalar1=1.0 / HD, scalar2=EPS,
                    op0=MULT, op1=ADD,
                )
                # rinv = rsqrt(aa) via Newton on DVE (keeps ScalarE on the
                # exp/tanh table only)
                rinv = small.tile([P, NG], F32, tag="rinv")
                I32 = mybir.dt.int32
                nc.vector.tensor_scalar(
                    out=rinv[:].bitcast(I32), in0=aa[:].bitcast(I32),
                    scalar1=1, scalar2=0,
                    op0=mybir.AluOpType.logical_shift_right,
                    op1=mybir.AluOpType.logical_shift_left)
                nc.vector.tensor_scalar(
                    out=rinv[:].bitcast(I32), in0=rinv[:].bitcast(I32),
                    scalar1=-1, scalar2=0x5F3759DF, op0=MULT, op1=ADD)
                t_n = small.tile([P, NG], F32, tag="t_n")
                for _ in range(2):
                    nc.vector.tensor_mul(t_n, rinv, rinv)
                    nc.vector.tensor_mul(t_n, t_n, aa)
                    nc.vector.tensor_scalar(
                        out=t_n, in0=t_n, scalar1=-0.5, scalar2=1.5,
                        op0=MULT, op1=ADD,
                    )
                    nc.vector.tensor_mul(rinv, rinv, t_n)
                # apply 1/rms (pre-rope; norm weight rides the rope tables)
                qkn = work.tile([P, EQK], F32, tag="qkn")
                rinv_ap = rinv[:]
                rinv_b = bass.AP(tensor=rinv_ap.tensor, offset=rinv_ap.offset,
                                 ap=[*rinv_ap.ap, [0, HD]])
                nc.vector.tensor_tensor(
                    out=qkn.rearrange("p (g d) -> p g d", d=HD),
                    in0=pp[:, 0:EQK].rearrange("p (g d) -> p g d", d=HD),
                    in1=rinv_b, op=MULT)
                # rope: out = qkn * cosw_dup + swap(qkn) * sinw_signed
                m1 = work.tile([P, EQK], F32, tag="sq")
                nc.gpsimd.tensor_mul(
                    m1[:, 0:EQ], qkn[:, 0:EQ],
                    bcast_groups(cosq[:, si, :], EQ // HD))
                nc.gpsimd.tensor_mul(
                    m1[:, EQ:EQK], qkn[:, EQ:EQK],
                    bcast_groups(cosk[:, si, :], EK // HD))
                xr = work.tile([P, EQK], F32, tag="xr")
                xrv = xr.rearrange("p (n two) -> p n two", two=2)
                qknv = qkn.rearrange("p (n two) -> p n two", two=2)

                def halfpairs(tbl, n):
                    # [p, n-group-bcast, 32 pairs] strided view of sin table
                    v = tbl[:, si, :].rearrange("p (n two) -> p n two", two=2)
                    return (
                        bass.AP(tensor=v.tensor, offset=v.offset,
                                ap=[v.ap[0], [0, n], *v.ap[1:-1]]),
                        bass.AP(tensor=v.tensor, offset=v.offset + 1,
                                ap=[v.ap[0], [0, n], *v.ap[1:-1]]),
                    )

                sq_e, sq_o = halfpairs(sinq, EQ // HD)
                sk_e, sk_o = halfpairs(sink, EK // HD)
                NQP = EQ // 2
                # xr_even = qkn_odd * sin_even ; xr_odd = qkn_even * sin_odd
                nc.vector.tensor_tensor(
                    out=xrv[:, 0:NQP, 0].rearrange("p (g n) -> p g n", g=EQ // HD),
                    in0=qknv[:, 0:NQP, 1].rearrange("p (g n) -> p g n", g=EQ // HD),
                    in1=sq_e, op=MULT)
                nc.vector.tensor_tensor(
                    out=xrv[:, 0:NQP, 1].rearrange("p (g n) -> p g n", g=EQ // HD),
                    in0=qknv[:, 0:NQP, 0].rearrange("p (g n) -> p g n", g=EQ // HD),
                    in1=sq_o, op=MULT)
                nc.vector.tensor_tensor(
                    out=xrv[:, NQP:, 0].rearrange("p (g n) -> p g n", g=EK // HD),
                    in0=qknv[:, NQP:, 1].rearrange("p (g n) -> p g n", g=EK // HD),
                    in1=sk_e, op=MULT)
                nc.vector.tensor_tensor(
                    out=xrv[:, NQP:, 1].rearrange("p (g n) -> p g n", g=EK // HD),
                    in0=qknv[:, NQP:, 0].rearrange("p (g n) -> p g n", g=EK // HD),
                    in1=sk_o, op=MULT)
                qkr = work.tile([P, EQK], F32R, tag="qkr")
                nc.gpsimd.tensor_add(qkr, m1, xr)
                # transpose q heads and k to [dim, s] (fp32r)
                pq = psB.tile([P, 512], F32, tag="B")
                pqr = pq[:].bitcast(F32R)
                for h in range(G):
                    nc.tensor.transpose(
                        pqr[:, h * P:(h + 1) * P], qkr[:, h * P:(h + 1) * P],
                        ident_r,
                    )
                nc.vector.tensor_copy(qT[si][:].rearrange("p g s -> p (g s)"), pq)
                pk = psB.tile([P, 512], F32, tag="B")
                pkr = pk[:].bitcast(F32R)
                nc.tensor.transpose(pkr[:, 0:P], qkr[:, EQ:EQK], ident_r)
                ksl = slice((si % 4) * P, (si % 4 + 1) * P)
                nc.vector.tensor_copy(kTg[si // 4][:, ksl], pk[:, 0:P])

            def scores_head(qb, h, t12):
                """scores + tanh + mask + exp for one head; returns e12,r1,r2"""
                L = (qb + 1) * P
                dg = qb * P
                for kc in range(0, L, 512):
                    w = min(512, L - kc)
                    sc = psA.tile([P, 1024], F32, tag="A")
                    sc2 = sc.rearrange("p (m c) -> p m c", m=2)
                    ci = kc // 512
                    has_dg = kc <= dg < kc + w
                    for br in range(2):
                        qh = br * HD
                        nc.tensor.matmul(
                            sc2[:, br, 0:w],
                            qT[qb][qh:qh + HD, h, :],
                            kTg[ci][qh:qh + HD, 0:w],
                            start=True, stop=not has_dg,
                        )
                        if has_dg:
                            # causal mask of the diagonal block, applied as
                            # a PE accumulation (I.T @ maskm = maskm)
                            nc.tensor.matmul(
                                sc2[:, br, dg - kc:dg - kc + P],
                                ident_r, maskm_r,
                                start=False, stop=True,
                            )
                    nc.scalar.activation(
                        t12[:, :, kc:kc + w], sc2[:, :, 0:w],
                        Tanh, scale=SCALE / CAP,
                    )
                e12 = work.tile([P, 2, S], BF16, tag="e12")
                r1 = small.tile([P, 1], F32, tag="r")
                r2 = small.tile([P, 1], F32, tag="r")
                nc.scalar.activation(
                    e12[:, 0, 0:L], t12[:, 0, 0:L], Exp, scale=CAP,
                    accum_out=r1)
                nc.scalar.activation(
                    e12[:, 1, 0:L], t12[:, 1, 0:L], Exp, scale=CAP,
                    accum_out=r2)
                return e12, r1, r2

            def attn_transpose(qb, h, e12, r1, r2, at_sb):
                """normalized diff attention, transposed:
                at = e1.T @ diag(1/r1) + e2.T @ diag(-lam/r2), bf16"""
                L = (qb + 1) * P
                r1i = small.tile([P, 1], F32, tag="r")
                nc.vector.reciprocal(r1i, r1)
                r2i = small.tile([P, 1], F32, tag="r")
                nc.vector.reciprocal(r2i, r2)
                nr2i = small.tile([P, 1], F32, tag="r")
                nc.vector.tensor_scalar(
                    out=nr2i, in0=r2i, scalar1=lam_sb[:, 0:1], scalar2=-1.0,
                    op0=MULT, op1=MULT,
                )
                diag1 = diagp.tile([P, P], BF16, tag="diag1")
                nc.vector.tensor_scalar_mul(diag1, ident_bf, r1i[:, 0:1])
                diag2 = diagp.tile([P, P], BF16, tag="diag2")
                nc.vector.tensor_scalar_mul(diag2, ident_bf, nr2i[:, 0:1])
                for kc in range(0, L, 512):
                    w = min(512, L - kc)
                    at4 = psAT.tile([P, 512], F32, tag="AT")
                    for kk in range(0, w, P):
                        sl = slice(kc + kk, kc + kk + P)
                        nc.tensor.matmul(
                            at4[:, kk:kk + P], e12[:, 0, sl], diag1,
                            start=(kk == 0), stop=False,
                        )
                    for kk in range(0, w, P):
                        sl = slice(kc + kk, kc + kk + P)
                        nc.tensor.matmul(
                            at4[:, kk:kk + P], e12[:, 1, sl], diag2,
                            start=False, stop=(kk + P >= w),
                        )
                    nc.vector.tensor_copy(at_sb[:, kc:kc + w], at4[:, 0:w])

            # ---- flat softstream over (qb, h): scores(qb, h) issues on
            # the PE before the post-work (at4/AV/O-proj) of the previous
            # head, including across qb boundaries, so the ScalarE tanh/exp
            # stream never starves at iteration tails ----
            es = {}
            ats = {}
            oTs = {}

            def do_scores(qb, h):
                t12 = t12p.tile([P, 2, S], F32, tag=f"t12_{h % 2}",
                                name=f"t12_{qb}_{h}")
                es[(qb, h)] = scores_head(qb, h, t12)

            def do_post(qb, h):
                nkb = qb + 1
                e12, r1, r2 = es.pop((qb, h))
                at_sb = atp.tile([P, S], BF16, tag=f"at{h % 2}",
                                 name=f"at_{qb}_{h}")
                attn_transpose(qb, h, e12, r1, r2, at_sb)
                ats[(qb, h % 2)] = at_sb
                if h % 2 == 1:
                    hp = h // 2
                    if qb not in oTs:
                        oTs[qb] = [
                            otp.tile([P, P], F32R, name=f"oT{qb}_{k}",
                                     tag=f"oT{k}") for k in range(2)]
                    po = psC.tile([P, P], F32, tag="C", name=f"po{qb}{hp}")
                    for kb in range(nkb):
                        sl = slice(kb * P, (kb + 1) * P)
                        nc.tensor.matmul(
                            po[0:HD, :], v_sb[kb], ats[(qb, 0)][:, sl],
                            start=(kb == 0), stop=(kb == nkb - 1),
                            tile_position=(0, 0),
                        )
                        nc.tensor.matmul(
                            po[HD:P, :], v_sb[kb], ats[(qb, 1)][:, sl],
                            start=(kb == 0), stop=(kb == nkb - 1),
                            tile_position=(0, 64),
                        )
                    nc.vector.tensor_copy(oTs[qb][hp], po)
                if h == G - 1:
                    oT = oTs.pop(qb)
                    y_sb = work.tile([P, D], F32, tag="y", name=f"y{qb}")
                    for ch in range(2):
                        py = psB.tile([P, 512], F32, tag="B", name=f"py{qb}{ch}")
                        sl = slice(ch * 512, (ch + 1) * 512)
                        nc.tensor.matmul(py, oT[0], wo_sb[:, 0, sl],
                                         start=True, stop=False)
                        nc.tensor.matmul(py, oT[1], wo_sb[:, 1, sl],
                                         start=False, stop=True)
                        nc.vector.tensor_copy(y_sb[:, sl], py)
                    nc.sync.dma_start(y_d[qb * P:(qb + 1) * P, :], y_sb)

            LOOKAHEAD = int(os.environ.get("K_LOOKAHEAD", "2"))
            for si in range(min(LOOKAHEAD, NSB)):
                phase1(si)
            pending = []
            next_p1 = min(LOOKAHEAD, NSB)
            for qb in range(NSB):
                for h in range(G):
                    do_scores(qb, h)
                    if pending:
                        do_post(*pending.pop(0))
                    pending.append((qb, h))
                    if h in (1, 3) and next_p1 < NSB and next_p1 <= qb + 4:
                        phase1(next_p1)
                        next_p1 += 1
            while pending:
                do_post(*pending.pop(0))

    nc.finalize()
    return nc


_NC = None


def _get_nc():
    global _NC
    if _NC is None:
        _NC = _build_nc()
    return _NC


def kernel(x, rope_freqs, wq, wk, wv, wo, q_norm_w, k_norm_w, diff_lambda):
    x = np.asarray(x, dtype=np.float32)
    rope_freqs = np.asarray(rope_freqs, dtype=np.float32)
    wq, wk, wv, wo = (np.asarray(a, dtype=np.float32) for a in (wq, wk, wv, wo))
    q_norm_w = np.asarray(q_norm_w, dtype=np.float32)
    k_norm_w = np.asarray(k_norm_w, dtype=np.float32)
    diff_lambda = np.asarray(diff_lambda, dtype=np.float32)

    cos = np.repeat(rope_freqs[:, :, 0], 2, axis=1).astype(np.float32)
    sin = np.repeat(rope_freqs[:, :, 1], 2, axis=1).astype(np.float32)
    sin_s = sin.copy()
    sin_s[:, 0::2] *= -1.0
    # norm weights folded into the rope tables:
    #   out0 = w0 x0 c - w1 x1 s = x0*(c w0) + swap(x)0*(sin_s0 * w1)
    #   out1 = w1 x1 c + w0 x0 s = x1*(c w1) + swap(x)1*(sin_s1 * w0)
    qw = np.asarray(q_norm_w)
    kw = np.asarray(k_norm_w)
    qw_sw = qw.reshape(-1, 2)[:, ::-1].reshape(-1)
    kw_sw = kw.reshape(-1, 2)[:, ::-1].reshape(-1)
    cosq = (cos * qw[None, :]).astype(np.float32)
    sinq = (sin_s * qw_sw[None, :]).astype(np.float32)
    cosk = (cos * kw[None, :]).astype(np.float32)
    sink = (sin_s * kw_sw[None, :]).astype(np.float32)

    in_maps = []
    for c in range(8):
        b, j = divmod(c, KV)
        w_all_t = np.zeros((D, EPAD), dtype=np.float32)
        w_all_t[:, 0:EALL] = np.concatenate(
            [
                wq[EQ * j:EQ * (j + 1), :],
                wk[EK * j:EK * (j + 1), :],
                wv[EV * j:EV * (j + 1), :],
            ],
            axis=0,
        ).T
        wo_t = np.ascontiguousarray(wo[:, 2 * P * j:2 * P * (j + 1)].T)
        in_maps.append(
            {
                "x": np.ascontiguousarray(x[b]),
                "w": w_all_t,
                "wo": wo_t,
                "cosq": cosq,
                "sinq": sinq,
                "cosk": cosk,
                "sink": sink,
                "lam": diff_lambda.reshape(1),
            }
        )

    nc = _get_nc()
    trace = os.environ.get("KERNEL_TRACE") == "1"
    res = run_bass_kernel_spmd(nc, in_maps, core_ids=list(range(8)), trace=trace)
    if trace and res.exec_time_ns is not None:
        print(f"HW exec time: {res.exec_time_ns} ns")

    out = np.zeros((B, S, D), dtype=np.float32)
    for c in range(8):
        b = c // KV
        out[b] += res.results[c]["y"]
    return out


# revision 27
# speedup vs baseline: 1.6339x; 1.0540x over previous
"""Trainium2 Bass kernel for nn_CausalAttention (diff-attention with QK-norm,
RoPE, GQA, tanh soft-cap, causal softmax).

Sharding: 8 cores = (batch b in {0,1}) x (kv-group j in {0..3}).
Each core handles one batch element and the 4 query heads of one kv head.

Perf strategy (vs the hi/lo-bf16 baseline):
  - all big GEMMs (x-transpose, projections, scores, O-proj) run in fp32r
    (1 cyc/row for moving dim >= 256, ~1.5e-4 rel err) instead of 3x
    bf16 hi/lo matmuls
  - RMSNorm rinv is applied pre-rope on DVE; q_norm_w/k_norm_w are folded
    into the rope cos/sin tables host-side (w varies per dim inside a rope
    pair, so it must ride the tables, not a post-rope scale)
  - softmax normalization (1/r1, -lambda/r2) is folded into the
    attn-transpose step: A^T = e1.T @ diag(1/r1) + e2.T @ diag(-lam/r2)
    as two accumulated regular bf16 matmuls (1 cyc/row); exp outputs bf16
  - AV runs bf16 (V eviction to bf16), O-proj fp32r
  - ScalarE does only tanh + exp (one act table, loaded once); rsqrt stays
    a DVE Newton chain so no table reloads ever happen
"""

import os
import sys

import numpy as np

if "/opt/trn_rl_repo" not in sys.path:
    sys.path.insert(0, "/opt/trn_rl_repo")

import concourse.bass as bass
import concourse.mybir as mybir
import concourse.tile as tile
from concourse import bacc
from concourse.bass_utils import run_bass_kernel_spmd
from concourse.masks import make_identity

B, S, D = 2, 2048, 1024
H, KV, HD = 16, 4, 64
G = H // KV          # q heads per kv head (= heads per core)
CAP = 50.0
EPS = 1e-6
SCALE = 1.0 / 8.0    # 1/sqrt(HD)
P = 128
NSB = S // P         # 16 s-blocks
EQ = G * 2 * HD      # 512 q-projection cols per core
EK = 2 * HD          # 128 k-projection cols per core
EV = HD              # 64  v-projection cols per core
EQK = EQ + EK        # 640 cols needing norm+rope
EALL = EQ + EK + EV  # 704 projection cols per core
EPAD = 768           # proj width padded to psum-bank-aligned 512+256 groups
NG = EQK // HD       # 10 rmsnorm groups
KT = D // P          # 8 contraction tiles
MASK_FILL = -1.0e4   # exp(50 * -1e4) == 0 in fp32

F32 = mybir.dt.float32
F32R = mybir.dt.float32r
F16 = mybir.dt.float16
BF16 = mybir.dt.bfloat16
MULT = mybir.AluOpType.mult
ADD = mybir.AluOpType.add
Tanh = mybir.ActivationFunctionType.Tanh
Exp = mybir.ActivationFunctionType.Exp


def _build_nc():
    nc = bacc.Bacc()
    x_d = nc.declare_dram_parameter("x", [S, D], F32R, isOutput=False)
    w_d = nc.declare_dram_parameter("w", [D, EPAD], F32R, isOutput=False)
    wo_d = nc.declare_dram_parameter("wo", [2 * P, D], F32R, isOutput=False)
    cosq_d = nc.declare_dram_parameter("cosq", [S, HD], F32, isOutput=False)
    sinq_d = nc.declare_dram_parameter("sinq", [S, HD], F32, isOutput=False)
    cosk_d = nc.declare_dram_parameter("cosk", [S, HD], F32, isOutput=False)
    sink_d = nc.declare_dram_parameter("sink", [S, HD], F32, isOutput=False)
    lam_d = nc.declare_dram_parameter("lam", [1], F32, isOutput=False)
    y_d = nc.declare_dram_parameter("y", [S, D], F32, isOutput=True)

    with tile.TileContext(nc) as tc:
        with (
            tc.tile_pool(name="singles", bufs=1) as singles,
            tc.tile_pool(name="persist", bufs=1) as persist,
            tc.tile_pool(name="work", bufs=2) as work,
            tc.tile_pool(name="t12p", bufs=1) as t12p,
            tc.tile_pool(name="atp", bufs=2) as atp,
            tc.tile_pool(name="small", bufs=6) as small,
            tc.tile_pool(name="diagp", bufs=2) as diagp,
            tc.tile_pool(name="otp", bufs=2) as otp,
            tc.tile_pool(name="psA", bufs=2, space="PSUM") as psA,
            tc.tile_pool(name="psAT", bufs=2, space="PSUM") as psAT,
            tc.tile_pool(name="psB", bufs=1, space="PSUM") as psB,
            tc.tile_pool(name="psC", bufs=1, space="PSUM") as psC,
        ):
            # ---- one-time setup ----
            w_t = []
            for t in range(KT):
                wt = singles.tile([P, EPAD], F32R, name=f"w{t}", tag=f"w{t}")
                eng = nc.sync if t % 2 == 0 else nc.scalar
                eng.dma_start(wt, w_d[t * P:(t + 1) * P, :])
                w_t.append(wt)
            cosq = singles.tile([P, NSB, HD], F32)
            nc.gpsimd.dma_start(cosq, cosq_d.rearrange("(n p) f -> p n f", p=P))
            sinq = singles.tile([P, NSB, HD], F32)
            nc.gpsimd.dma_start(sinq, sinq_d.rearrange("(n p) f -> p n f", p=P))
            cosk = singles.tile([P, NSB, HD], F32)
            nc.gpsimd.dma_start(cosk, cosk_d.rearrange("(n p) f -> p n f", p=P))
            sink = singles.tile([P, NSB, HD], F32)
            nc.gpsimd.dma_start(sink, sink_d.rearrange("(n p) f -> p n f", p=P))
            wo_sb = singles.tile([P, 2, D], F32R)
            nc.gpsimd.dma_start(wo_sb, wo_d.rearrange("(t p) e -> p t e", p=P))

            setup_scr = work.tile([P, EALL], F32, tag="ppsb", name="setupscr")
            ident_f = setup_scr[:, 0:P]
            make_identity(nc, ident_f)
            ident_r = singles.tile([P, P], F32R)
            nc.vector.tensor_copy(ident_r, ident_f)
            ident_bf = singles.tile([P, P], BF16)
            nc.gpsimd.tensor_copy(ident_bf, ident_f)
            from concourse.masks import make_causal_mask
            maskm_f = setup_scr[:, P:2 * P]
            make_causal_mask(nc, maskm_f, mask_val=-1.0e8)
            maskm_r = singles.tile([P, P], F32R)
            nc.vector.tensor_copy(maskm_r, maskm_f)

            def part_bcast(handle):
                ap = handle[:]
                return bass.AP(tensor=ap.tensor, offset=ap.offset,
                               ap=[[0, P], *ap.ap])

            lam_sb = singles.tile([P, 1], F32)
            nc.gpsimd.dma_start(lam_sb, part_bcast(lam_d))

            # persistent per-core activation storage
            v_sb = [persist.tile([P, EV], BF16, name=f"v{i}", tag=f"v{i}")
                    for i in range(NSB)]
            qT = [persist.tile([P, G, P], F32R, name=f"qT{i}", tag=f"qT{i}")
                  for i in range(NSB)]
            kTg = [persist.tile([P, 4 * P], F32R, name=f"kT{i}", tag=f"kT{i}")
                   for i in range(NSB // 4)]

            def bcast_groups(src2d, n):
                return bass.AP(
                    tensor=src2d.tensor,
                    offset=src2d.offset,
                    ap=[src2d.ap[0], [0, n], src2d.ap[-1]],
                )

            p1state = {}
            p1x = {}

            def phase1_dma(si):
                x_sb = work.tile([P, D], F32R, tag="x")
                nc.sync.dma_start(x_sb, x_d[si * P:(si + 1) * P, :])
                p1x[si] = x_sb

            def phase1(si):
                """projections + rmsnorm + rope + transposes for s-block si"""
                x_sb = p1x.pop(si)
                # transpose x block: [s,d] -> [d,s] (fp32r, 1.5 cyc/row)
                xT = work.tile([P, KT, P], F32R, tag="xT")
                for half in range(2):
                    pt = psB.tile([P, 512], F32, tag="B")
                    ptr = pt[:].bitcast(F32R)
                    for t in range(4):
                        tt = 4 * half + t
                        nc.tensor.transpose(
                            ptr[:, t * P:(t + 1) * P],
                            x_sb[:, tt * P:(tt + 1) * P], ident_r,
                        )
                    nc.vector.tensor_copy(
                        xT[:, 4 * half:4 * half + 4, :].rearrange(
                            "p a b -> p (a b)"), pt)
                # projections: fp32r, two 352-wide psum groups
                pp = psA.tile([P, 1024], F32, tag="A")
                for co, cw in ((0, 512), (512, 256)):
                    for t in range(KT):
                        nc.tensor.matmul(
                            pp[:, co:co + cw], xT[:, t, :],
                            w_t[t][:, co:co + cw],
                            start=(t == 0), stop=(t == KT - 1),
                        )
                # v: eviction to bf16
                nc.vector.tensor_copy(v_sb[si], pp[:, EQK:EALL])
                sq = work.tile([P, EQK], F32, tag="sq")
                nc.scalar.activation(
                    sq, pp[:, 0:EQK], mybir.ActivationFunctionType.Square)
                ssq = small.tile([P, NG], F32, tag="ssq")
                nc.vector.tensor_reduce(
                    ssq, sq.rearrange("p (g d) -> p g d", d=HD),
                    axis=mybir.AxisListType.X, op=ADD,
                )
                aa = small.tile([P, NG], F32, tag="aa")
                nc.gpsimd.tensor_scalar(
                    out=aa, in0=ssq, scalar1=1.0 / HD, scalar2=EPS,
                    op0=MULT, op1=ADD,
                )
                # rinv = rsqrt(aa) via Newton on DVE (keeps ScalarE on the
                # exp/tanh table only)
                rinv = small.tile([P, NG], F32, tag="rinv")
                I32 = mybir.dt.int32
                nc.vector.tensor_scalar(
                    out=rinv[:].bitcast(I32), in0=aa[:].bitcast(I32),
                    scalar1=1, scalar2=0,
                    op0=mybir.AluOpType.logical_shift_right,
                    op1=mybir.AluOpType.logical_shift_left)
                nc.vector.tensor_scalar(
                    out=rinv[:].bitcast(I32), in0=rinv[:].bitcast(I32),
                    scalar1=-1, scalar2=0x5F3759DF, op0=MULT, op1=ADD)
                t_n = small.tile([P, NG], F32, tag="t_n")
                for _ in range(2):
                    nc.vector.tensor_mul(t_n, rinv, rinv)
                    nc.vector.tensor_mul(t_n, t_n, aa)
                    nc.vector.tensor_scalar(
                        out=t_n, in0=t_n, scalar1=-0.5, scalar2=1.5,
                        op0=MULT, op1=ADD,
                    )
                    nc.vector.tensor_mul(rinv, rinv, t_n)
                # apply 1/rms (pre-rope; norm weight rides the rope tables)
                qkn = work.tile([P, EQK], F32, tag="qkn")
                rinv_ap = rinv[:]
                rinv_b = bass.AP(tensor=rinv_ap.tensor, offset=rinv_ap.offset,
                                 ap=[*rinv_ap.ap, [0, HD]])
                nc.vector.tensor_tensor(
                    out=qkn.rearrange("p (g d) -> p g d", d=HD),
                    in0=pp[:, 0:EQK].rearrange("p (g d) -> p g d", d=HD),
                    in1=rinv_b, op=MULT)
                # rope: out = qkn * cosw_dup + swap(qkn) * sinw_signed
                m1 = work.tile([P, EQK], F32, tag="sq")
                nc.gpsimd.tensor_mul(
                    m1[:, 0:EQ], qkn[:, 0:EQ],
                    bcast_groups(cosq[:, si, :], EQ // HD))
                nc.gpsimd.tensor_mul(
                    m1[:, EQ:EQK], qkn[:, EQ:EQK],
                    bcast_groups(cosk[:, si, :], EK // HD))
                xr = work.tile([P, EQK], F32, tag="xr")
                xrv = xr.rearrange("p (n two) -> p n two", two=2)
                qknv = qkn.rearrange("p (n two) -> p n two", two=2)

                def halfpairs(tbl, n):
                    # [p, n-group-bcast, 32 pairs] strided view of sin table
                    v = tbl[:, si, :].rearrange("p (n two) -> p n two", two=2)
                    return (
                        bass.AP(tensor=v.tensor, offset=v.offset,
                                ap=[v.ap[0], [0, n], *v.ap[1:-1]]),
                        bass.AP(tensor=v.tensor, offset=v.offset + 1,
                                ap=[v.ap[0], [0, n], *v.ap[1:-1]]),
                    )

                sq_e, sq_o = halfpairs(sinq, EQ // HD)
                sk_e, sk_o = halfpairs(sink, EK // HD)
                NQP = EQ // 2
                # xr_even = qkn_odd * sin_even ; xr_odd = qkn_even * sin_odd
                nc.vector.tensor_tensor(
                    out=xrv[:, 0:NQP, 0].rearrange("p (g n) -> p g n", g=EQ // HD),
                    in0=qknv[:, 0:NQP, 1].rearrange("p (g n) -> p g n", g=EQ // HD),
                    in1=sq_e, op=MULT)
                nc.vector.tensor_tensor(
                    out=xrv[:, 0:NQP, 1].rearrange("p (g n) -> p g n", g=EQ // HD),
                    in0=qknv[:, 0:NQP, 0].rearrange("p (g n) -> p g n", g=EQ // HD),
                    in1=sq_o, op=MULT)
                nc.vector.tensor_tensor(
                    out=xrv[:, NQP:, 0].rearrange("p (g n) -> p g n", g=EK // HD),
                    in0=qknv[:, NQP:, 1].rearrange("p (g n) -> p g n", g=EK // HD),
                    in1=sk_e, op=MULT)
                nc.vector.tensor_tensor(
                    out=xrv[:, NQP:, 1].rearrange("p (g n) -> p g n", g=EK // HD),
                    in0=qknv[:, NQP:, 0].rearrange("p (g n) -> p g n", g=EK // HD),
                    in1=sk_o, op=MULT)
                qkr = work.tile([P, EQK], F32R, tag="qkr")
                nc.gpsimd.tensor_add(qkr, m1, xr)
                p1state[si] = qkr

            def phase1b(si):
                qkr = p1state.pop(si)
                # transpose q heads and k to [dim, s] (fp32r)
                pq = psB.tile([P, 512], F32, tag="B")
                pqr = pq[:].bitcast(F32R)
                for h in range(G):
                    nc.tensor.transpose(
                        pqr[:, h * P:(h + 1) * P], qkr[:, h * P:(h + 1) * P],
                        ident_r,
                    )
                nc.vector.tensor_copy(qT[si][:].rearrange("p g s -> p (g s)"), pq)
                pk = psB.tile([P, 512], F32, tag="B")
                pkr = pk[:].bitcast(F32R)
                nc.tensor.transpose(pkr[:, 0:P], qkr[:, EQ:EQK], ident_r)
                ksl = slice((si % 4) * P, (si % 4 + 1) * P)
                nc.vector.tensor_copy(kTg[si // 4][:, ksl], pk[:, 0:P])

            def scores_head(qb, h, t12):
                """scores + tanh + mask + exp for one head; returns e12,r1,r2"""
                L = (qb + 1) * P
                dg = qb * P
                for kc in range(0, L, 512):
                    w = min(512, L - kc)
                    sc = psA.tile([P, 1024], F32, tag="A")
                    sc2 = sc.rearrange("p (m c) -> p m c", m=2)
                    ci = kc // 512
                    has_dg = kc <= dg < kc + w
                    for br in range(2):
                        qh = br * HD
                        nc.tensor.matmul(
                            sc2[:, br, 0:w],
                            qT[qb][qh:qh + HD, h, :],
                            kTg[ci][qh:qh + HD, 0:w],
                            start=True, stop=not has_dg,
                        )
                        if has_dg:
                            # causal mask of the diagonal block, applied as
                            # a PE accumulation (I.T @ maskm = maskm)
                            nc.tensor.matmul(
                                sc2[:, br, dg - kc:dg - kc + P],
                                ident_r, maskm_r,
                                start=False, stop=True,
                            )
                    nc.scalar.activation(
                        t12[:, :, kc:kc + w], sc2[:, :, 0:w],
                        Tanh, scale=SCALE / CAP,
                    )
                e12 = work.tile([P, 2, S], BF16, tag="e12")
                r1 = small.tile([P, 1], F32, tag="r")
                r2 = small.tile([P, 1], F32, tag="r")
                nc.scalar.activation(
                    e12[:, 0, 0:L], t12[:, 0, 0:L], Exp, scale=CAP,
                    accum_out=r1)
                nc.scalar.activation(
                    e12[:, 1, 0:L], t12[:, 1, 0:L], Exp, scale=CAP,
                    accum_out=r2)
                return e12, r1, r2

            def attn_transpose(qb, h, e12, r1, r2, at_sb):
                """normalized diff attention, transposed:
                at = e1.T @ diag(1/r1) + e2.T @ diag(-lam/r2), bf16"""
                L = (qb + 1) * P
                r1i = small.tile([P, 1], F32, tag="r")
                nc.vector.reciprocal(r1i, r1)
                r2i = small.tile([P, 1], F32, tag="r")
                nc.vector.reciprocal(r2i, r2)
                nr2i = small.tile([P, 1], F32, tag="r")
                nc.vector.tensor_scalar(
                    out=nr2i, in0=r2i, scalar1=lam_sb[:, 0:1], scalar2=-1.0,
                    op0=MULT, op1=MULT,
                )
                diag1 = diagp.tile([P, P], BF16, tag="diag1")
                nc.vector.tensor_scalar_mul(diag1, ident_bf, r1i[:, 0:1])
                diag2 = diagp.tile([P, P], BF16, tag="diag2")
                nc.vector.tensor_scalar_mul(diag2, ident_bf, nr2i[:, 0:1])
                for kc in range(0, L, 512):
                    w = min(512, L - kc)
                    at4 = psAT.tile([P, 512], F32, tag="AT")
                    for kk in range(0, w, P):
                        sl = slice(kc + kk, kc + kk + P)
                        nc.tensor.matmul(
                            at4[:, kk:kk + P], e12[:, 0, sl], diag1,
                            start=(kk == 0), stop=False,
                        )
                    for kk in range(0, w, P):
                        sl = slice(kc + kk, kc + kk + P)
                        nc.tensor.matmul(
                            at4[:, kk:kk + P], e12[:, 1, sl], diag2,
                            start=False, stop=(kk + P >= w),
                        )
                    nc.vector.tensor_copy(at_sb[:, kc:kc + w], at4[:, 0:w])

            # ---- flat softstream over (qb, h): scores(qb, h) issues on
            # the PE before the post-work (at4/AV/O-proj) of the previous
            # head, including across qb boundaries, so the ScalarE tanh/exp
            # stream never starves at iteration tails ----
            es = {}
            ats = {}
            oTs = {}

            def do_scores(qb, h):
                t12 = t12p.tile([P, 2, S], F16, tag=f"t12_{h % 2}",
                                name=f"t12_{qb}_{h}")
                es[(qb, h)] = scores_head(qb, h, t12)

            def do_post(qb, h):
                nkb = qb + 1
                e12, r1, r2 = es.pop((qb, h))
                at_sb = atp.tile([P, S], BF16, tag=f"at{h % 2}",
                                 name=f"at_{qb}_{h}")
                attn_transpose(qb, h, e12, r1, r2, at_sb)
                ats[(qb, h % 2)] = at_sb
                if h % 2 == 1:
                    hp = h // 2
                    if qb not in oTs:
                        oTs[qb] = [
                            otp.tile([P, P], F32R, name=f"oT{qb}_{k}",
                                     tag=f"oT{k}") for k in range(2)]
                    po = psC.tile([P, P], F32, tag="C", name=f"po{qb}{hp}")
                    for kb in range(nkb):
                        sl = slice(kb * P, (kb + 1) * P)
                        nc.tensor.matmul(
                            po[0:HD, :], v_sb[kb], ats[(qb, 0)][:, sl],
                            start=(kb == 0), stop=(kb == nkb - 1),
                            tile_position=(0, 0),
                        )
                        nc.tensor.matmul(
                            po[HD:P, :], v_sb[kb], ats[(qb, 1)][:, sl],
                            start=(kb == 0), stop=(kb == nkb - 1),
                            tile_position=(0, 64),
                        )
                    nc.vector.tensor_copy(oTs[qb][hp], po)
                if h == G - 1:
                    oT = oTs.pop(qb)
                    y_sb = work.tile([P, D], F32, tag="y", name=f"y{qb}")
                    for ch in range(2):
                        py = psB.tile([P, 512], F32, tag="B", name=f"py{qb}{ch}")
                        sl = slice(ch * 512, (ch + 1) * 512)
                        nc.tensor.matmul(py, oT[0], wo_sb[:, 0, sl],
                                         start=True, stop=False)
                        nc.tensor.matmul(py, oT[1], wo_sb[:, 1, sl],
                                         start=False, stop=True)
                        nc.vector.tensor_copy(y_sb[:, sl], py)
                    nc.sync.dma_start(y_d[qb * P:(qb + 1) * P, :], y_sb)

            QB_ORDER = [0, 2] + list(range(4, NSB)) + [1, 3]
            NPRE = 4
            for si in range(min(NPRE, NSB)):
                phase1_dma(si)
                phase1(si)
                phase1b(si)
            pending = []
            next_p1 = min(NPRE, NSB)
            next_p1b = next_p1
            for qb in QB_ORDER:
                for h in range(G):
                    do_scores(qb, h)
                    if pending:
                        do_post(*pending.pop(0))
                    pending.append((qb, h))
                    if h == 0 and next_p1 < NSB and next_p1 <= qb + 4:
                        phase1_dma(next_p1)
                    if h == 1 and next_p1 < NSB and next_p1 <= qb + 4:
                        phase1(next_p1)
                        next_p1 += 1
                    if h == 3 and next_p1b < next_p1 - 1 and next_p1b < NSB:
                        phase1b(next_p1b)
                        next_p1b += 1
            while next_p1b < NSB:
                phase1b(next_p1b)
                next_p1b += 1
            while pending:
                do_post(*pending.pop(0))

    nc.finalize()
    return nc


_NC = None


def _get_nc():
    global _NC
    if _NC is None:
        _NC = _build_nc()
    return _NC


def kernel(x, rope_freqs, wq, wk, wv, wo, q_norm_w, k_norm_w, diff_lambda):
    x = np.asarray(x, dtype=np.float32)
    rope_freqs = np.asarray(rope_freqs, dtype=np.float32)
    wq, wk, wv, wo = (np.asarray(a, dtype=np.float32) for a in (wq, wk, wv, wo))
    q_norm_w = np.asarray(q_norm_w, dtype=np.float32)
    k_norm_w = np.asarray(k_norm_w, dtype=np.float32)
    diff_lambda = np.asarray(diff_lambda, dtype=np.float32)

    cos = np.repeat(rope_freqs[:, :, 0], 2, axis=1).astype(np.float32)
    sin = np.repeat(rope_freqs[:, :, 1], 2, axis=1).astype(np.float32)
    sin_s = sin.copy()
    sin_s[:, 0::2] *= -1.0
    # norm weights folded into the rope tables:
    #   out0 = w0 x0 c - w1 x1 s = x0*(c w0) + swap(x)0*(sin_s0 * w1)
    #   out1 = w1 x1 c + w0 x0 s = x1*(c w1) + swap(x)1*(sin_s1 * w0)
    qw = np.asarray(q_norm_w)
    kw = np.asarray(k_norm_w)
    qw_sw = qw.reshape(-1, 2)[:, ::-1].reshape(-1)
    kw_sw = kw.reshape(-1, 2)[:, ::-1].reshape(-1)
    cosq = (cos * qw[None, :]).astype(np.float32)
    sinq = (sin_s * qw_sw[None, :]).astype(np.float32)
    cosk = (cos * kw[None, :]).astype(np.float32)
    sink = (sin_s * kw_sw[None, :]).astype(np.float32)

    in_maps = []
    for c in range(8):
        b, j = divmod(c, KV)
        w_all_t = np.zeros((D, EPAD), dtype=np.float32)
        w_all_t[:, 0:EALL] = np.concatenate(
            [
                wq[EQ * j:EQ * (j + 1), :],
                wk[EK * j:EK * (j + 1), :],
                wv[EV * j:EV * (j + 1), :],
            ],
            axis=0,
        ).T
        wo_t = np.ascontiguousarray(wo[:, 2 * P * j:2 * P * (j + 1)].T)
        in_maps.append(
            {
                "x": np.ascontiguousarray(x[b]),
                "w": w_all_t,
                "wo": wo_t,
                "cosq": cosq,
                "sinq": sinq,
                "cosk": cosk,
                "sink": sink,
                "lam": diff_lambda.reshape(1),
            }
        )

    nc = _get_nc()
    trace = os.environ.get("KERNEL_TRACE") == "1"
    res = run_bass_kernel_spmd(nc, in_maps, core_ids=list(range(8)), trace=trace)
    if trace and res.exec_time_ns is not None:
        print(f"HW exec time: {res.exec_time_ns} ns")

    out = np.zeros((B, S, D), dtype=np.float32)
    for c in range(8):
        b = c // KV
        out[b] += res.results[c]["y"]
    return out
